# revision 1
# baseline (speedup 1.0000x reference)
"""AxialLinearAttention Trainium2 kernel (v2).

Data-parallel over batch across 8 NeuronCores (all math is batch-local).
Per core: feature-transposed activation layout (X^T: features on
partitions, tokens on the free dim); every projection is a dense
(128x128)@(128x512) bf16 matmul chain; linear attention is computed per
(head, 128-token chunk) as bf16 score matmuls with the axial group
structure applied as a constant block-diagonal mask fused into the PSUM
eviction.

v2 over v1:
 - all 8 weight matrices live resident in SBUF as bf16 (128KB/part),
   loaded once in the prologue via gpsimd casting DMAs -> no
   per-supertile weight traffic, no SP descriptor storm
 - activations bf16 end to end (x is cast f32->bf16 by the input DMA);
   all transposes are bf16 (1 cyc/row); one eviction per PSUM tile
 - residual stream updated in place (xt += delta) so ant/freq blocks
   share one tile set
 - elu1(x) = min(exp(x), 1) + relu(x): exp on scalar, relu alternating
   scalar/vector, min/add combine on vector (gpsimd has no PSUM port and
   no TensorScalar opcode on TRN2)
 - write_out(s-1) deferred into supertile s's q phase; scores pipelined
   LOOK=2 head-pairs ahead of AV consumption
"""

import os
import sys

sys.path.insert(0, "/opt/trn_rl_repo")

import numpy as np

import concourse.bacc as bacc
import concourse.bass as bass
import concourse.mybir as mybir
import concourse.tile as tile
from concourse.masks import make_identity

F32 = mybir.dt.float32
F32R = mybir.dt.float32r
BF16 = mybir.dt.bfloat16
AF = mybir.ActivationFunctionType
ALU = mybir.AluOpType

B, FG, ANT, D = 256, 4, 32, 1024
H, DK = 16, 64
NCORES = 8
P = 128
NPT = D // P  # 8 feature partition-tiles

W_NAMES = [
    "ant_q_w", "ant_k_w", "ant_v_w", "ant_out_w",
    "freq_q_w", "freq_k_w", "freq_v_w", "freq_out_w",
]


def _emit_kernel(nc, tc, ctx, BC):
    T = BC * FG * ANT          # tokens per core
    ST = min(512, T)           # tokens per super-tile
    NST = T // ST
    SL = ST // 128             # 128-token chunks per super-tile
    HM = SL * 128              # scores width per head-parity

    XF32 = False
    x_d = nc.dram_tensor("x", [T, D], F32R if XF32 else F32,
                         kind="ExternalInput").ap()
    w_d = {n: nc.dram_tensor(n, [D, D], F32, kind="ExternalInput").ap()
           for n in W_NAMES}
    out_d = nc.dram_tensor("out", [T, D], F32, kind="ExternalOutput").ap()

    # ---- pools ----
    const_pool = ctx.enter_context(tc.tile_pool(name="const", bufs=1))
    wres = ctx.enter_context(tc.tile_pool(name="wres", bufs=1))
    big = ctx.enter_context(tc.tile_pool(name="big", bufs=1))
    xt_pool = ctx.enter_context(tc.tile_pool(name="xtp", bufs=2))
    xstage_p = ctx.enter_context(tc.tile_pool(name="xstage", bufs=1))
    ostage_p = ctx.enter_context(tc.tile_pool(name="ostage", bufs=2))
    tmp_p = ctx.enter_context(tc.tile_pool(name="tmp", bufs=2))
    sm_p = ctx.enter_context(tc.tile_pool(name="smp", bufs=12))
    ps_pj = ctx.enter_context(tc.tile_pool(name="ps_pj", bufs=3, space="PSUM"))
    ps_sc = ctx.enter_context(tc.tile_pool(name="ps_sc", bufs=3, space="PSUM"))
    ps_vv = ctx.enter_context(tc.tile_pool(name="ps_vv", bufs=1, space="PSUM"))
    ps_tb = ctx.enter_context(tc.tile_pool(name="ps_tb", bufs=1, space="PSUM"))

    # ---- constants ----
    ident = const_pool.tile([P, P], F32)
    make_identity(nc, ident)
    identb = const_pool.tile([P, P], BF16)
    nc.scalar.activation(identb, ident, AF.Copy)
    identr = None
    if XF32:
        identr = const_pool.tile([P, P], F32R)
        nc.scalar.activation(identr, ident, AF.Copy)

    # base masks, one 128-token chunk wide, replicated across HM columns.
    # ant: tokens grouped in contiguous blocks of 32 (groups = (b, fg))
    mant = const_pool.tile([P, P], F32)
    nc.gpsimd.memset(mant, 0.0)
    for g in range(4):
        nc.gpsimd.memset(mant[32 * g:32 * g + 32, 32 * g:32 * g + 32], 1.0)
    # freq: groups are (b, ant): l' interacts with l iff l' % 32 == l % 32
    mfrq = const_pool.tile([P, P], F32)
    nc.gpsimd.memset(mfrq, 0.0)
    for a in range(4):
        for b2 in range(4):
            nc.vector.tensor_copy(
                mfrq[32 * a:32 * a + 32, 32 * b2:32 * b2 + 32], ident[0:32, 0:32])
    mask_ant = const_pool.tile([P, HM], BF16)
    mask_freq = const_pool.tile([P, HM], BF16)
    for rep in range(HM // P):
        nc.vector.tensor_copy(mask_ant[:, rep * P:(rep + 1) * P], mant)
        nc.vector.tensor_copy(mask_freq[:, rep * P:(rep + 1) * P], mfrq)

    # ---- x stage DMAs for supertile 0 first (same gpsimd queue as the
    # weight loads; these four must land before the weight burst) ----
    xstage = {}

    def stage_x(s):
        for sl in range(SL):
            if XF32:
                t = xstage_p.tile([P, D], F32R, tag=f"xs{sl}")
                nc.sync.dma_start(
                    t, x_d[s * ST + sl * P:s * ST + (sl + 1) * P, :])
            else:
                t = xstage_p.tile([P, D], BF16, tag=f"xs{sl}")
                nc.gpsimd.dma_start(
                    t, x_d[s * ST + sl * P:s * ST + (sl + 1) * P, :])
            xstage[sl] = t

    stage_x(0)

    # ---- resident weights: two casting DMAs per matrix (half the i-tiles
    # each), emitted in first-supertile consumption order so the initial
    # q/k/v fills start as soon as their operand halves land ----
    w_sb = {}
    for n in W_NAMES:
        w_sb[n] = wres.tile([P, NPT * D], BF16, tag=n, name=n)
    for n in W_NAMES:
        w = w_sb[n]
        for ih in range(2):
            h = NPT // 2
            nc.gpsimd.dma_start(
                w[:, ih * h * D:(ih + 1) * h * D].rearrange(
                    "p (i j) -> p i j", i=h),
                w_d[n][ih * h * P:(ih + 1) * h * P, :].rearrange(
                    "(i p) j -> p i j", p=P))

    def wsl(n, i, j0, j1):
        """lhsT block: rows = feature tile i, cols j0:j1 of W[n]."""
        return w_sb[n][:, i * D + j0:i * D + j1]

    # ================= per-super-tile emission =================

    def transpose_in():
        xt = []
        for i in range(NPT):
            if XF32:
                tp = ps_tb.tile([P, ST], F32R, tag="tb")
                srcb = None
            else:
                tp = ps_tb.tile([P, ST], BF16, tag="tb")
            for sl in range(SL):
                nc.tensor.transpose(
                    tp[:, sl * P:(sl + 1) * P],
                    xstage[sl][:, i * P:(i + 1) * P],
                    identr if XF32 else identb)
            xti = xt_pool.tile([P, ST], BF16, tag=f"xt{i}", name=f"xt{i}")
            tpv = tp.bitcast(F32) if XF32 else tp
            if i % 2 == 0:
                nc.vector.tensor_copy(xti, tpv)
            else:
                nc.scalar.activation(xti, tpv, AF.Copy)
            xt.append(xti)
        return xt

    def proj_T(wn, src, elu, dst_tag, interleave=None):
        """out^T[j] = sum_i W[i,j]^T @ src^T[i]."""
        dst = []
        pending = list(interleave or [])
        for j in range(NPT):
            ps = ps_pj.tile([P, ST], F32, tag="pj")
            for i in range(NPT):
                nc.tensor.matmul(
                    ps, lhsT=wsl(wn, i, j * P, (j + 1) * P), rhs=src[i],
                    start=(i == 0), stop=(i == NPT - 1))
            o = big.tile([P, ST], BF16, tag=f"{dst_tag}{j}",
                         name=f"{dst_tag}{j}")
            if elu:
                # elu1(x) = min(exp(x), 1) + relu(x); relu alternates
                # engines so neither scalar nor vector saturates
                e = tmp_p.tile([P, ST], BF16, tag="e")
                nc.scalar.activation(e, ps, AF.Exp)
                rl = tmp_p.tile([P, ST], BF16, tag="r")
                if j % 2 == 0:
                    nc.vector.tensor_scalar_max(rl, ps, 0.0)
                else:
                    nc.scalar.activation(rl, ps, AF.Relu)
                nc.vector.scalar_tensor_tensor(
                    o, e, 1.0, rl, op0=ALU.min, op1=ALU.add)
            else:
                nc.scalar.activation(o, ps, AF.Copy)
            dst.append(o)
            if pending:
                pending.pop(0)()
        while pending:
            pending.pop(0)()
        return dst

    def proj_V(wn, src, interleave=None):
        """X^T-stationary projection -> V in natural (token, feature) layout.

        `interleave`: list of thunks (score emissions) run one per group so
        their DVE mask-evictions drain during the v fills.
        """
        v = [big.tile([P, D], BF16, tag=f"v{sl}", name=f"v{sl}")
             for sl in range(SL)]
        pending = list(interleave or [])
        for j2 in range(2):
            for sl in range(SL):
                ps = ps_vv.tile([P, HM], F32, tag="vv")
                for i in range(NPT):
                    nc.tensor.matmul(
                        ps,
                        lhsT=src[i][:, sl * P:(sl + 1) * P],
                        rhs=wsl(wn, i, j2 * 512, (j2 + 1) * 512),
                        start=(i == 0), stop=(i == NPT - 1))
                nc.scalar.activation(
                    v[sl][:, j2 * 512:(j2 + 1) * 512], ps, AF.Copy)
                if pending:
                    pending.pop(0)()
        while pending:
            pending.pop(0)()
        return v

    def make_scores(qt, kt, mask, sms):
        """Emit the masked score computation for one (head-pair, parity)."""

        def emit(hp, par):
            off = 64 * par
            sp = ps_sc.tile([P, HM], F32, tag="sc")
            for c in range(SL):
                nc.tensor.matmul(
                    sp[:, c * P:(c + 1) * P],
                    lhsT=kt[hp][off:off + 64, c * P:(c + 1) * P],
                    rhs=qt[hp][off:off + 64, c * P:(c + 1) * P],
                    start=True, stop=True)
            sm = sm_p.tile([P, HM], BF16, tag="sm")
            nc.vector.tensor_tensor(sm, sp, mask, op=ALU.mult)
            sms.setdefault(hp, []).append(sm)

        return emit

    def attention(qt, kt, v, sms, scores, pre):
        """AV per head-pair; scores for hp >= pre are emitted LOOK ahead."""
        at = []
        LOOK = 2
        for hp in range(pre, min(pre + LOOK, NPT)):
            scores(hp, 0)
            scores(hp, 1)
        for hp in range(NPT):
            nxt = hp + LOOK
            if pre <= nxt < NPT:
                scores(nxt, 0)
                scores(nxt, 1)
            ap_ = ps_pj.tile([P, ST], F32, tag="pj")
            for par in range(2):
                off = 64 * par
                for c in range(SL):
                    nc.tensor.matmul(
                        ap_[off:off + 64, c * P:(c + 1) * P],
                        lhsT=v[c][:, hp * P + off:hp * P + off + 64],
                        rhs=sms[hp][par][:, c * P:(c + 1) * P],
                        start=True, stop=True)
            del sms[hp]
            o = big.tile([P, ST], BF16, tag=f"at{hp}", name=f"at{hp}")
            if hp % 2 == 0:
                nc.vector.tensor_copy(o, ap_)
            else:
                nc.scalar.activation(o, ap_, AF.Copy)
            at.append(o)
        return at

    def outproj_residual(wn, at, res, interleave=None):
        """res^T[j] += W_o[:,j]^T @ A^T  (in-place residual update)."""
        pending = list(interleave or [])
        for j in range(NPT):
            ps = ps_pj.tile([P, ST], F32, tag="pj")
            for i in range(NPT):
                nc.tensor.matmul(
                    ps, lhsT=wsl(wn, i, j * P, (j + 1) * P), rhs=at[i],
                    start=(i == 0), stop=(i == NPT - 1))
            nc.vector.tensor_add(res[j], ps, res[j])
            # thunks read res[<j'] of THIS supertile: only safe for groups
            # whose fin tiles are already updated; caller pads with None
            if pending:
                t = pending.pop(0)
                if t is not None:
                    t()
        for t in pending:
            if t is not None:
                t()

    def write_out_thunks(fin, s):
        def emit(sl, jh):
            tp = ps_tb.tile([P, ST], BF16, tag="tb")
            for j4 in range(4):
                j = jh * 4 + j4
                nc.tensor.transpose(
                    tp[:, j4 * P:(j4 + 1) * P],
                    fin[j][:, sl * P:(sl + 1) * P], identb)
            ost = ostage_p.tile([P, 512], F32, tag="os")
            if jh == 0:
                nc.scalar.activation(ost, tp, AF.Copy)
            else:
                nc.vector.tensor_copy(ost, tp)
            nc.sync.dma_start(
                out_d[s * ST + sl * P:s * ST + (sl + 1) * P,
                      jh * 512:(jh + 1) * 512], ost)

        return [(lambda sl=sl, jh=jh: emit(sl, jh))
                for sl in range(SL) for jh in range(2)]

    def write_out(fin, s):
        for t in write_out_thunks(fin, s):
            t()

    # ================= main loop =================
    # write_out(s-1) is deferred into supertile s's ant-q phase so the PE
    # has dense matmul work at the supertile boundary and the write-out /
    # transpose eviction burst spreads into the q/k phase slack.
    REP = int(os.environ.get("K_REPEAT", "1"))  # timing experiments only
    prev = None
    for s_ in range(NST * REP):
        s = s_ % NST
        xt = transpose_in()
        # prefetch next supertile's tokens (dep: xstage consumed just above)
        if s_ + 1 < NST * REP:
            stage_x((s_ + 1) % NST)
        for blk, mask in (("ant", mask_ant), ("freq", mask_freq)):
            wt_ = (write_out_thunks(*prev)
                   if blk == "ant" and prev is not None else None)
            qt = proj_T(f"{blk}_q_w", xt, True, "qt", interleave=wt_)
            kt = proj_T(f"{blk}_k_w", xt, True, "kt")
            sms = {}
            scores = make_scores(qt, kt, mask, sms)
            PRE = 0
            thunks = [
                (lambda hp=hp, par=par: scores(hp, par))
                for hp in range(PRE) for par in range(2)]
            v = proj_V(f"{blk}_v_w", xt, interleave=thunks)
            at = attention(qt, kt, v, sms, scores, PRE)
            last = (blk == "freq" and s_ == NST * REP - 1)
            if last:
                th = write_out_thunks(xt, s)
                # jh=0 thunks (indices 0,2,4,6) only need res[0..3]: fire
                # them after outproj j=4..7; jh=1 thunks after the loop
                order = ([None] * 4 + [th[i] for i in (0, 2, 4, 6)]
                         + [th[i] for i in (1, 3, 5, 7)])
                outproj_residual(f"{blk}_out_w", at, xt,
                                 interleave=order)
            else:
                outproj_residual(f"{blk}_out_w", at, xt)
        prev = (xt, s)
    if prev is not None and not (NST * REP >= 1):
        write_out(*prev)


def build(BC):
    from contextlib import ExitStack

    nc = bacc.Bacc("TRN2", target_bir_lowering=False, debug=False)
    with tile.TileContext(nc) as tc:
        with ExitStack() as ctx:
            _emit_kernel(nc, tc, ctx, BC)
    nc.compile()
    return nc


_CACHE = {}
last_results = None


def kernel(x, **inputs):
    """Full (unsharded) inputs -> full output. Shards batch across 8 cores."""
    global last_results
    from concourse.bass_utils import run_bass_kernel_spmd

    x = np.ascontiguousarray(np.asarray(x), dtype=np.float32)
    BC = B // NCORES
    if "nc" not in _CACHE:
        _CACHE["nc"] = build(BC)
    nc = _CACHE["nc"]

    weights = {n: np.ascontiguousarray(np.asarray(inputs[n]), dtype=np.float32)
               for n in W_NAMES}
    in_maps = []
    for k in range(NCORES):
        m = {"x": x[k * BC:(k + 1) * BC].reshape(BC * FG * ANT, D)}
        m.update(weights)
        in_maps.append(m)

    res = run_bass_kernel_spmd(nc, in_maps, core_ids=list(range(NCORES)))
    last_results = res
    out = np.empty((B, FG * ANT, D), dtype=np.float32)
    for k in range(NCORES):
        out[k * BC:(k + 1) * BC] = res.results[k]["out"].reshape(BC, FG * ANT, D)
    return out



# revision 4
# speedup vs baseline: 1.0184x; 1.0184x over previous
"""AxialLinearAttention Trainium2 kernel (v3).

Data-parallel over batch across 8 NeuronCores (all math is batch-local).
Per core: feature-transposed activation layout (X^T: features on
partitions, tokens on the free dim); every projection is a dense
(128x128)@(128x512) bf16 matmul chain; linear attention is computed per
(head, 128-token chunk) as bf16 score matmuls with the axial group
structure applied as a constant block-diagonal mask fused into the PSUM
eviction.

v3 over v2:
 - x arrives pre-transposed (and pre-cast bf16) from the host as
   xT [D, T]; DMA lands tokens directly in the resident xt tiles ->
   no PE in-transposes, no transpose evictions, no staging tiles
 - output leaves feature-major as bf16 [D, T] straight from the
   residual tiles (host transposes back); no PE out-transposes, no
   ostage evictions. The residual is bf16 anyway, so bit-identical.
 - weights arrive bf16 from the host (half the prologue DMA traffic)
 - elu1(x) = min(exp(x), 1) + relu(x): exp on scalar, relu alternating
   scalar/vector, min/add combine on vector
 - scores pipelined LOOK=2 head-pairs ahead of AV consumption
"""

import os
import sys

sys.path.insert(0, "/opt/trn_rl_repo")

import numpy as np

import concourse.bacc as bacc
import concourse.bass as bass
import concourse.mybir as mybir
import concourse.tile as tile

F32 = mybir.dt.float32
BF16 = mybir.dt.bfloat16
AF = mybir.ActivationFunctionType
ALU = mybir.AluOpType

B, FG, ANT, D = 256, 4, 32, 1024
H, DK = 16, 64
NCORES = 8
P = 128
NPT = D // P  # 8 feature partition-tiles

W_NAMES = [
    "ant_q_w", "ant_k_w", "ant_v_w", "ant_out_w",
    "freq_q_w", "freq_k_w", "freq_v_w", "freq_out_w",
]


def _emit_kernel(nc, tc, ctx, BC):
    T = BC * FG * ANT          # tokens per core
    ST = min(512, T)           # tokens per super-tile
    NST = T // ST
    SL = ST // 128             # 128-token chunks per super-tile
    HM = SL * 128              # scores width per head-parity

    x_d = nc.dram_tensor("xt", [D, T], BF16, kind="ExternalInput").ap()
    w_d = {n: nc.dram_tensor(n, [D, D], BF16, kind="ExternalInput").ap()
           for n in W_NAMES}
    out_d = nc.dram_tensor("out", [D, T], BF16, kind="ExternalOutput").ap()

    # ---- pools ----
    const_pool = ctx.enter_context(tc.tile_pool(name="const", bufs=1))
    wres = ctx.enter_context(tc.tile_pool(name="wres", bufs=1))
    big = ctx.enter_context(tc.tile_pool(name="big", bufs=1))
    xt_pool = ctx.enter_context(tc.tile_pool(name="xtp", bufs=2))
    tmp_p = ctx.enter_context(tc.tile_pool(name="tmp", bufs=2))
    sm_p = ctx.enter_context(tc.tile_pool(name="smp", bufs=12))
    ps_pj = ctx.enter_context(tc.tile_pool(name="ps_pj", bufs=4, space="PSUM"))
    ps_sc = ctx.enter_context(tc.tile_pool(name="ps_sc", bufs=3, space="PSUM"))
    ps_vv = ctx.enter_context(tc.tile_pool(name="ps_vv", bufs=1, space="PSUM"))

    # ---- constant score masks ----
    from concourse.masks import make_identity
    ident = const_pool.tile([P, P], F32)
    make_identity(nc, ident)
    # base masks, one 128-token chunk wide, replicated across HM columns.
    # ant: tokens grouped in contiguous blocks of 32 (groups = (b, fg))
    mant = const_pool.tile([P, P], F32)
    nc.gpsimd.memset(mant, 0.0)
    for g in range(4):
        nc.gpsimd.memset(mant[32 * g:32 * g + 32, 32 * g:32 * g + 32], 1.0)
    # freq: groups are (b, ant): l' interacts with l iff l' % 32 == l % 32
    mfrq = const_pool.tile([P, P], F32)
    nc.gpsimd.memset(mfrq, 0.0)
    for a in range(4):
        for b2 in range(4):
            nc.vector.tensor_copy(
                mfrq[32 * a:32 * a + 32, 32 * b2:32 * b2 + 32],
                ident[0:32, 0:32])
    mask_ant = const_pool.tile([P, HM], BF16)
    mask_freq = const_pool.tile([P, HM], BF16)
    for rep in range(HM // P):
        nc.vector.tensor_copy(mask_ant[:, rep * P:(rep + 1) * P], mant)
        nc.vector.tensor_copy(mask_freq[:, rep * P:(rep + 1) * P], mfrq)

    # ---- x tiles for supertile 0 (sync queue: runs in parallel with the
    # weight burst on the gpsimd queue) ----
    xts = {}

    def stage_x(s, slot):
        xt = []
        for i in range(NPT):
            t = xt_pool.tile([P, ST], BF16, tag=f"xt{i}", name=f"xt{i}_{slot}")
            nc.sync.dma_start(
                t, x_d[i * P:(i + 1) * P, s * ST:(s + 1) * ST])
            xt.append(t)
        xts[slot] = xt

    stage_x(0, 0)

    # ---- resident weights: two DMAs per matrix (half the i-tiles each),
    # emitted in first-supertile consumption order ----
    w_sb = {}
    for n in W_NAMES:
        w_sb[n] = wres.tile([P, NPT * D], BF16, tag=n, name=n)
    for n in W_NAMES:
        w = w_sb[n]
        for ih in range(2):
            h = NPT // 2
            nc.gpsimd.dma_start(
                w[:, ih * h * D:(ih + 1) * h * D].rearrange(
                    "p (i j) -> p i j", i=h),
                w_d[n][ih * h * P:(ih + 1) * h * P, :].rearrange(
                    "(i p) j -> p i j", p=P))

    def wsl(n, i, j0, j1):
        """lhsT block: rows = feature tile i, cols j0:j1 of W[n]."""
        return w_sb[n][:, i * D + j0:i * D + j1]

    # ================= per-super-tile emission =================

    def proj_T(wn, src, elu, dst_tag):
        """out^T[j] = sum_i W[i,j]^T @ src^T[i]."""
        dst = []
        for j in range(NPT):
            ps = ps_pj.tile([P, ST], F32, tag="pj")
            for i in range(NPT):
                nc.tensor.matmul(
                    ps, lhsT=wsl(wn, i, j * P, (j + 1) * P), rhs=src[i],
                    start=(i == 0), stop=(i == NPT - 1))
            o = big.tile([P, ST], BF16, tag=f"{dst_tag}{j}",
                         name=f"{dst_tag}{j}")
            if elu:
                # elu1(x) = min(exp(x), 1) + relu(x); relu alternates
                # engines so neither scalar nor vector saturates
                e = tmp_p.tile([P, ST], BF16, tag="e")
                nc.scalar.activation(e, ps, AF.Exp)
                rl = tmp_p.tile([P, ST], BF16, tag="r")
                if j % 2 == 0:
                    nc.vector.tensor_scalar_max(rl, ps, 0.0)
                else:
                    nc.scalar.activation(rl, ps, AF.Relu)
                nc.vector.scalar_tensor_tensor(
                    o, e, 1.0, rl, op0=ALU.min, op1=ALU.add)
            else:
                nc.scalar.activation(o, ps, AF.Copy)
            dst.append(o)
        return dst

    def proj_V(wn, src, interleave=None):
        """X^T-stationary projection -> V in natural (token, feature) layout.

        `interleave`: list of thunks (score emissions) run one per group so
        their DVE mask-evictions drain during the v fills.
        """
        v = [big.tile([P, D], BF16, tag=f"v{sl}", name=f"v{sl}")
             for sl in range(SL)]
        pending = list(interleave or [])
        for j2 in range(2):
            for sl in range(SL):
                ps = ps_vv.tile([P, HM], F32, tag="vv")
                for i in range(NPT):
                    nc.tensor.matmul(
                        ps,
                        lhsT=src[i][:, sl * P:(sl + 1) * P],
                        rhs=wsl(wn, i, j2 * 512, (j2 + 1) * 512),
                        start=(i == 0), stop=(i == NPT - 1))
                nc.scalar.activation(
                    v[sl][:, j2 * 512:(j2 + 1) * 512], ps, AF.Copy)
                if pending:
                    pending.pop(0)()
        while pending:
            pending.pop(0)()
        return v

    def make_scores(qt, kt, mask, sms):
        """Emit the masked score computation for one (head-pair, parity)."""

        def emit(hp, par):
            off = 64 * par
            sp = ps_sc.tile([P, HM], F32, tag="sc")
            for c in range(SL):
                nc.tensor.matmul(
                    sp[:, c * P:(c + 1) * P],
                    lhsT=kt[hp][off:off + 64, c * P:(c + 1) * P],
                    rhs=qt[hp][off:off + 64, c * P:(c + 1) * P],
                    start=True, stop=True)
            sm = sm_p.tile([P, HM], BF16, tag="sm")
            nc.vector.tensor_tensor(sm, sp, mask, op=ALU.mult)
            sms.setdefault(hp, []).append(sm)

        return emit

    def attention(qt, kt, v, sms, scores, pre):
        """AV per head-pair; scores for hp >= pre are emitted LOOK ahead."""
        at = []
        LOOK = 2
        for hp in range(pre, min(pre + LOOK, NPT)):
            scores(hp, 0)
            scores(hp, 1)
        for hp in range(NPT):
            nxt = hp + LOOK
            if pre <= nxt < NPT:
                scores(nxt, 0)
                scores(nxt, 1)
            ap_ = ps_pj.tile([P, ST], F32, tag="pj")
            for par in range(2):
                off = 64 * par
                for c in range(SL):
                    nc.tensor.matmul(
                        ap_[off:off + 64, c * P:(c + 1) * P],
                        lhsT=v[c][:, hp * P + off:hp * P + off + 64],
                        rhs=sms[hp][par][:, c * P:(c + 1) * P],
                        start=True, stop=True)
            del sms[hp]
            o = big.tile([P, ST], BF16, tag=f"at{hp}", name=f"at{hp}")
            if hp % 2 == 0:
                nc.vector.tensor_copy(o, ap_)
            else:
                nc.scalar.activation(o, ap_, AF.Copy)
            at.append(o)
        return at

    def outproj_residual(wn, at, res, writeout=None):
        """res^T[j] += W_o[:,j]^T @ A^T  (in-place residual update).

        writeout: supertile index -> after each res[j] update, DMA it out.
        """
        for j in range(NPT):
            ps = ps_pj.tile([P, ST], F32, tag="pj")
            for i in range(NPT):
                nc.tensor.matmul(
                    ps, lhsT=wsl(wn, i, j * P, (j + 1) * P), rhs=at[i],
                    start=(i == 0), stop=(i == NPT - 1))
            nc.vector.tensor_add(res[j], ps, res[j])
            if writeout is not None:
                s = writeout
                nc.sync.dma_start(
                    out_d[j * P:(j + 1) * P, s * ST:(s + 1) * ST], res[j])

    # ================= main loop =================
    REP = int(os.environ.get("K_REPEAT", "1"))  # timing experiments only
    for s_ in range(NST * REP):
        s = s_ % NST
        slot = s_ % 2
        xt = xts[slot]
        # prefetch next supertile's tokens into the other buffer slot
        if s_ + 1 < NST * REP:
            stage_x((s_ + 1) % NST, 1 - slot)
        for blk, mask in (("ant", mask_ant), ("freq", mask_freq)):
            qt = proj_T(f"{blk}_q_w", xt, True, "qt")
            kt = proj_T(f"{blk}_k_w", xt, True, "kt")
            sms = {}
            scores = make_scores(qt, kt, mask, sms)
            v = proj_V(f"{blk}_v_w", xt)
            at = attention(qt, kt, v, sms, scores, 0)
            outproj_residual(
                f"{blk}_out_w", at, xt,
                writeout=(s if blk == "freq" else None))


def build(BC):
    from contextlib import ExitStack

    nc = bacc.Bacc("TRN2", target_bir_lowering=False, debug=False)
    with tile.TileContext(nc) as tc:
        with ExitStack() as ctx:
            _emit_kernel(nc, tc, ctx, BC)
    nc.compile()
    return nc


_CACHE = {}
last_results = None


def _prep_core_inputs(x, weights_bf16, k, BC):
    import ml_dtypes
    xk = x[k * BC:(k + 1) * BC].reshape(BC * FG * ANT, D)
    m = {"xt": np.ascontiguousarray(xk.T).astype(ml_dtypes.bfloat16)}
    m.update(weights_bf16)
    return m


def kernel(x, **inputs):
    """Full (unsharded) inputs -> full output. Shards batch across 8 cores."""
    global last_results
    import ml_dtypes
    from concourse.bass_utils import run_bass_kernel_spmd

    x = np.ascontiguousarray(np.asarray(x), dtype=np.float32)
    BC = B // NCORES
    if "nc" not in _CACHE:
        _CACHE["nc"] = build(BC)
    nc = _CACHE["nc"]

    weights = {n: np.asarray(inputs[n]).astype(ml_dtypes.bfloat16)
               for n in W_NAMES}
    in_maps = [_prep_core_inputs(x, weights, k, BC) for k in range(NCORES)]

    res = run_bass_kernel_spmd(nc, in_maps, core_ids=list(range(NCORES)))
    last_results = res
    out = np.empty((B, FG * ANT, D), dtype=np.float32)
    for k in range(NCORES):
        out[k * BC:(k + 1) * BC] = (
            res.results[k]["out"].astype(np.float32).T
            .reshape(BC, FG * ANT, D))
    return out
